# revision 17
# baseline (speedup 1.0000x reference)
"""Trainium2 Bass kernel for a 2-layer GraphConv + linear head (GCN-style).

Distribution: nodes (and their incident edges, by destination) are
partitioned across 8 NeuronCores. Weights are replicated. The per-layer
node-feature tables are exchanged with AllGather collectives.

Math (matches the reference):
    norm = clip(out_degree, 1)^-0.5           # per node, from src counts
    Y    = ((X * norm) @ w1)                  # layer1 matmul first (256>128)
    Z1   = segment_sum(Y[src] -> dst)
    H1   = relu(Z1 * norm + b1);  G = H1 * norm
    Z2   = segment_sum(G[src] -> dst)
    H2   = relu((Z2 @ w2) * norm + b2)
    OUT  = H2 @ w3.T + b3
    return (OUT, OUT)

Layout / precision notes:
  * X is pre-scaled by norm, pre-transposed, and cast to fp16 on the
    host; phase A uses X^T tiles as the *stationary* matmul operand and
    emits Y node-major (fp16) directly, so the y-table write is a
    contiguous DMA. The gather tables, weights, and one-hot matrices are
    all fp16 (halves DMA/collective bytes and PE moving-column time);
    accumulation stays fp32 in PSUM.
  * Layer 1 accumulates node-major: base windows use the small static
    ones matrices as the stationary operand (4 wide matmuls per dst
    block instead of 16 skinny ones), and the feature-major overflow
    side-accumulator is folded in with transpose matmuls that accumulate
    into the same PSUM group. The G table write is then contiguous.
  * Layer 2 stays feature-major (its w2/w3 matmuls want that
    orientation), and the final output is written feature-major; the
    host transposes it back.

The scatter-add (segment_sum) runs on the TensorEngine:
  * base pass: every (dst, chunk) gets SLOTS fixed gather slots; a window
    of 128 tokens covers 128/SLOTS dsts. Pad slots gather a zeroed table
    row.
  * overflow pass: edges beyond the fixed slots use data-driven one-hot
    windows (iota-vs-dstloc is_equal on the VectorEngine, fp16).

Gathers use the custom SWDGE dma_gather instruction (int16 indices), so
the gather table is split into 4 address chunks (< 32768 rows each); each
rank's AllGather contribution carries trailing zero rows so every chunk
contains a zero row for padding tokens. Gather instructions rotate over
4 SWDGE queues.
"""

import numpy as np

import concourse.bass as bass
import concourse.bacc as bacc
import concourse.tile as tile
import concourse.mybir as mybir
from concourse import bass_utils

F32 = mybir.dt.float32
F16 = mybir.dt.float16
I16 = mybir.dt.int16

NC_CORES = 8
NCHUNK = 4
SLOTS = 4          # base gather slots per (dst, chunk)
WIN = 128          # tokens per scatter window (PE contraction dim)
WPD = WIN // SLOTS # dsts covered by one base window
NQ = 4             # SWDGE queues for gathers
GCAP = 2048        # max tokens per gather instruction; needs
                   # single_packet=False (single_packet coalesces one
                   # 64-descriptor-max SDMA packet per engine)


class Plan:
    """Host-side preprocessing: slot/overflow assignment, index arrays,
    static (shared-across-cores) schedule."""

    def __init__(self, n_nodes, e_subgraph, tile_d=512, zpad=44):
        N = n_nodes
        assert N % NC_CORES == 0
        self.N = N
        self.NLOC = N // NC_CORES
        self.ZPAD = zpad
        self.CONTRIB = self.NLOC + zpad
        assert (NC_CORES * self.CONTRIB) % NCHUNK == 0
        self.CHUNK = NC_CORES * self.CONTRIB // NCHUNK
        assert self.CHUNK <= 32767, self.CHUNK
        self.TILE_D = tile_d
        self.NT = -(-self.NLOC // tile_d)
        self.PADLOC = self.NT * tile_d

        src = np.asarray(e_subgraph[0], dtype=np.int64)
        dst = np.asarray(e_subgraph[1], dtype=np.int64)

        deg = np.bincount(src, minlength=N).astype(np.float32)
        self.norm = np.clip(deg, 1.0, None) ** -0.5

        srow = (src // self.NLOC) * self.CONTRIB + (src % self.NLOC)
        schunk = srow // self.CHUNK
        slidx = (srow - schunk * self.CHUNK).astype(np.int64)
        owner = dst // self.NLOC
        dloc = dst % self.NLOC

        # per-core edge assignment
        per_core = []
        for c in range(NC_CORES):
            sel = owner == c
            dl, ch, li = dloc[sel], schunk[sel], slidx[sel]
            order = np.lexsort((ch, dl))
            dl, ch, li = dl[order], ch[order], li[order]
            key = dl * NCHUNK + ch
            is_new = np.r_[True, key[1:] != key[:-1]] if len(key) else np.array([], bool)
            grp_id = np.cumsum(is_new) - 1 if len(key) else key
            if len(key):
                grp_start = np.flatnonzero(is_new)
                rank = np.arange(len(key)) - grp_start[grp_id]
            else:
                rank = key
            per_core.append((dl, ch, li, rank))

        zero_lidx = self.NLOC  # first zero row inside every chunk

        # base arrays + overflow lists
        NT, TILE_D, CHUNK = self.NT, self.TILE_D, self.CHUNK
        base = [np.full((NT, NCHUNK, TILE_D * SLOTS), zero_lidx, np.int64)
                for _ in range(NC_CORES)]
        ovf = [[[([], []) for _ in range(NCHUNK)] for _ in range(NT)]
               for _ in range(NC_CORES)]
        for c in range(NC_CORES):
            dl, ch, li, rank = per_core[c]
            t = dl // TILE_D
            din = dl - t * TILE_D
            bm = rank < SLOTS
            base[c][t[bm], ch[bm], din[bm] * SLOTS + rank[bm]] = li[bm]
            om = ~bm
            for tt, cc, dd, ll in zip(t[om], ch[om], din[om], li[om]):
                ovf[c][tt][cc][0].append(ll)
                ovf[c][tt][cc][1].append(dd)

        # static overflow window counts (max over cores)
        self.nw = np.zeros((NT, NCHUNK), np.int64)
        for t in range(NT):
            for cc in range(NCHUNK):
                mx = max(len(ovf[c][t][cc][0]) for c in range(NC_CORES))
                self.nw[t, cc] = -(-mx // WIN) if mx else 0

        # token stream: per tile, per chunk: [base TILE_D*SLOTS][ovf nw*WIN]
        self.seg = np.zeros((NT, NCHUNK), np.int64)
        for t in range(NT):
            for cc in range(NCHUNK):
                self.seg[t, cc] = TILE_D * SLOTS + self.nw[t, cc] * WIN
        self.tile_tokens = self.seg.sum(axis=1)
        self.tile_groups = self.tile_tokens // WIN
        self.tot_cols = int(self.tile_tokens.sum()) // 16
        self.nw_tot = int(self.nw.sum())

        # build per-core idx / dstloc arrays
        self.idx = np.zeros((NC_CORES, 128, self.tot_cols), np.int16)
        self.dstloc = np.full((NC_CORES, 128, max(self.nw_tot, 1)), -1.0, np.float32)
        for c in range(NC_CORES):
            col = 0
            w_i = 0
            for t in range(NT):
                for cc in range(NCHUNK):
                    toks = np.full(int(self.seg[t, cc]), zero_lidx, np.int64)
                    toks[:TILE_D * SLOTS] = base[c][t, cc]
                    ll, dd = ovf[c][t][cc]
                    if len(ll):
                        toks[TILE_D * SLOTS:TILE_D * SLOTS + len(ll)] = ll
                    seg = int(self.seg[t, cc])
                    wrapped = toks.astype(np.int16).reshape(seg // 16, 16).T
                    self.idx[c, :, col:col + seg // 16] = np.tile(wrapped, (8, 1))
                    col += seg // 16
                    for j in range(int(self.nw[t, cc])):
                        sl = dd[j * WIN:(j + 1) * WIN]
                        if len(sl):
                            self.dstloc[c, :len(sl), w_i] = sl
                        w_i += 1
            assert col == self.tot_cols

        # norm broadcast [128, PADLOC] per core (feature-major layers)
        self.normb = np.ones((NC_CORES, 128, self.PADLOC), np.float32)
        for c in range(NC_CORES):
            nl = self.norm[c * self.NLOC:(c + 1) * self.NLOC]
            self.normb[c, :, :self.NLOC] = nl[None, :]
        # per-node norm columns [128, NT*4] for node-major layer 1
        self.normc = np.ones((NC_CORES, 128, self.NT * 4), np.float32)
        for c in range(NC_CORES):
            nl = np.ones(self.PADLOC, np.float32)
            nl[:self.NLOC] = self.norm[c * self.NLOC:(c + 1) * self.NLOC]
            self.normc[c] = nl.reshape(self.NT * 4, 128).T

    def consts(self):
        iota = np.broadcast_to(
            np.arange(self.TILE_D, dtype=np.float32), (128, self.TILE_D)).copy()
        onesb = np.zeros((128, WPD), np.float16)
        for tk in range(WIN):
            onesb[tk, tk // SLOTS] = 1.0
        # node-major grouped base: onesb4[:, j, :] maps token tk of window
        # (4b + j) to dst column j*WPD + tk//SLOTS of dst block b.
        onesb4 = np.zeros((128, 4, 128), np.float16)
        for j in range(4):
            for tk in range(WIN):
                onesb4[tk, j, j * WPD + tk // SLOTS] = 1.0
        ident = np.eye(128, dtype=np.float32)
        return iota, onesb, onesb4, ident


def build_nc(plan: Plan, din, dh, dout):
    """Emit the bass program (shared SPMD across all cores)."""
    p = plan
    nc = bacc.Bacc("TRN2", target_bir_lowering=False, debug=False,
                   num_devices=NC_CORES, num_swdge_queues=NQ)

    # pre-transposed, norm-scaled features [din, NLOC] (fp16)
    featsT = nc.dram_tensor("featsT", [din, p.NLOC], F16, kind="ExternalInput")
    w1_d = nc.dram_tensor("w1", [din, dh], F16, kind="ExternalInput")
    w2_d = nc.dram_tensor("w2", [dh, dh], F16, kind="ExternalInput")
    w3t_d = nc.dram_tensor("w3t", [dh, dout], F16, kind="ExternalInput")
    b1bc_d = nc.dram_tensor("b1bc", [128, dh], F32, kind="ExternalInput")
    b2_d = nc.dram_tensor("b2", [dh, 1], F32, kind="ExternalInput")
    b3_d = nc.dram_tensor("b3", [dout, 1], F32, kind="ExternalInput")
    normb_d = nc.dram_tensor("normb", [128, p.PADLOC], F32, kind="ExternalInput")
    normc_d = nc.dram_tensor("normc", [128, p.NT * 4], F32, kind="ExternalInput")
    idx_d = nc.dram_tensor("idx", [128, p.tot_cols], I16, kind="ExternalInput")
    dstloc_d = nc.dram_tensor("dstloc", [128, max(p.nw_tot, 1)], F32,
                              kind="ExternalInput")
    negd_d = nc.dram_tensor("negd", [128, max(p.nw_tot, 1)], F32,
                            kind="ExternalInput")
    iota_d = nc.dram_tensor("iota", [128, p.TILE_D], F32, kind="ExternalInput")
    onesb_d = nc.dram_tensor("onesb", [128, WPD], F16, kind="ExternalInput")
    onesb4_d = nc.dram_tensor("onesb4", [128, 4, 128], F16, kind="ExternalInput")
    ident_d = nc.dram_tensor("ident", [128, 128], F32, kind="ExternalInput")
    out_d = nc.dram_tensor("outT", [dout, p.PADLOC], F32, kind="ExternalOutput")

    y_loc = nc.dram_tensor("y_loc", [p.CONTRIB, dh], F16)
    g_loc = nc.dram_tensor("g_loc", [p.CONTRIB, dh], F16)
    t_y = nc.dram_tensor("t_y", [NC_CORES * p.CONTRIB, dh], F16,
                         addr_space="Shared")
    t_g = nc.dram_tensor("t_g", [NC_CORES * p.CONTRIB, dh], F16,
                         addr_space="Shared")

    kt = din // 128  # K-tiles for layer-1 matmul
    gmax = int(p.tile_groups.max())
    cols_max = int(p.tile_tokens.max()) // 16
    qctr = [0]  # rotating SWDGE queue

    with tile.TileContext(nc) as tc:
        with (
            tc.tile_pool(name="const", bufs=1) as cp,
            tc.tile_pool(name="xt", bufs=3) as xtp,
            tc.tile_pool(name="yt", bufs=3) as ytp,
            tc.tile_pool(name="nb", bufs=2) as nbp,
            tc.tile_pool(name="gath", bufs=3) as gp,
            tc.tile_pool(name="idxp", bufs=2) as ixp,
            tc.tile_pool(name="oh", bufs=2) as ohp,
            tc.tile_pool(name="mid", bufs=2) as midp,
            tc.tile_pool(name="psA", bufs=1, space="PSUM") as psA,
            tc.tile_pool(name="psB", bufs=2, space="PSUM") as psB,
            tc.tile_pool(name="psC", bufs=2, space="PSUM") as psC,
            tc.tile_pool(name="psD", bufs=1, space="PSUM") as psD,
            tc.tile_pool(name="psF", bufs=2, space="PSUM") as psF,
        ):
            # ---- constants ----
            w1_sb = cp.tile([128, kt, dh], F16)
            for k in range(kt):
                nc.sync.dma_start(w1_sb[:, k, :], w1_d[k * 128:(k + 1) * 128, :])
            w2_sb = cp.tile([128, dh], F16)
            nc.sync.dma_start(w2_sb[:], w2_d[:, :])
            w3t_sb = cp.tile([128, dout], F16)
            nc.sync.dma_start(w3t_sb[:], w3t_d[:, :])
            b1bc_sb = cp.tile([128, dh], F32)
            nc.sync.dma_start(b1bc_sb[:], b1bc_d[:, :])
            b2_sb = cp.tile([dh, 1], F32)
            nc.sync.dma_start(b2_sb[:], b2_d[:, :])
            b3_sb = cp.tile([dout, 1], F32)
            nc.sync.dma_start(b3_sb[:], b3_d[:, :])
            iota_sb = cp.tile([128, p.TILE_D], F32)
            nc.sync.dma_start(iota_sb[:], iota_d[:, :])
            onesb_sb = cp.tile([128, WPD], F16)
            nc.sync.dma_start(onesb_sb[:], onesb_d[:, :])
            onesb4_sb = cp.tile([128, 4, 128], F16)
            nc.sync.dma_start(onesb4_sb[:], onesb4_d[:, :])
            ident_sb = cp.tile([128, 128], F32)
            nc.sync.dma_start(ident_sb[:], ident_d[:, :])
            normc_sb = cp.tile([128, p.NT * 4], F32)
            nc.sync.dma_start(normc_sb[:], normc_d[:, :])
            dstloc_sb = cp.tile([128, max(p.nw_tot, 1)], F32)
            nc.sync.dma_start(dstloc_sb[:], dstloc_d[:, :])
            negd_sb = cp.tile([128, max(p.nw_tot, 1)], F32)
            nc.sync.dma_start(negd_sb[:], negd_d[:, :])
            zeros_sb = cp.tile([128, dh], F16)
            nc.vector.memset(zeros_sb[:], 0.0)

            # ---- phase A: Y = (X*norm) @ w1, node-major via X^T stationary ----
            for t in range(p.NT):
                n0 = t * p.TILE_D
                nr = min(p.TILE_D, p.NLOC - n0)
                if nr <= 0:
                    break
                xt = xtp.tile([128, kt, p.TILE_D], F16)
                for k in range(kt):
                    nc.sync.dma_start(xt[:, k, :nr],
                                      featsT[k * 128:(k + 1) * 128, n0:n0 + nr])
                ysb = ytp.tile([128, 4, dh], F16)
                ps4 = psA.tile([128, 4, dh], F32, space="PSUM")
                for b in range(4):
                    nb0 = b * 128
                    nbw = min(128, nr - nb0)
                    if nbw <= 0:
                        break
                    for k in range(kt):
                        nc.tensor.matmul(ps4[:nbw, b, :],
                                         xt[:, k, nb0:nb0 + nbw],
                                         w1_sb[:, k, :],
                                         start=(k == 0), stop=(k == kt - 1))
                    nc.scalar.copy(ysb[:nbw, b, :], ps4[:nbw, b, :])
                    nc.sync.dma_start(y_loc[n0 + nb0:n0 + nb0 + nbw, :],
                                      ysb[:nbw, b, :])
            # zero pad rows of the contribution
            nc.sync.dma_start(y_loc[p.NLOC:p.CONTRIB, :], zeros_sb[:p.ZPAD, :])

            nc.gpsimd.collective_compute(
                "AllGather", mybir.AluOpType.bypass,
                ins=[y_loc.ap().opt()], outs=[t_y.ap().opt()],
                replica_groups=[list(range(NC_CORES))],
            )

            # ---- aggregation layers ----
            def one_hot(w_i):
                # one-hot [128, TILE_D] fp16 for overflow window w_i;
                # alternate Vector is_equal / Scalar relu(1-(iota-d)^2)
                oh = ohp.tile([128, p.TILE_D], F16, tag="oh")
                if w_i % 2 == 0:
                    nc.vector.tensor_scalar(
                        out=oh[:], in0=iota_sb[:],
                        scalar1=dstloc_sb[:, w_i:w_i + 1], scalar2=None,
                        op0=mybir.AluOpType.is_equal)
                else:
                    sq = ohp.tile([128, p.TILE_D], F32, tag="ohsq")
                    nc.scalar.activation(
                        sq[:], iota_sb[:],
                        mybir.ActivationFunctionType.Square,
                        bias=negd_sb[:, w_i:w_i + 1])
                    nc.scalar.activation(
                        oh[:], sq[:],
                        mybir.ActivationFunctionType.Relu,
                        bias=1.0, scale=-1.0)
                return oh

            def agg_layer(table, layer):
                col0 = 0
                w_i0 = 0
                for t in range(p.NT):
                    d0 = t * p.TILE_D
                    nd = min(p.TILE_D, p.NLOC - d0)
                    cols_t = int(p.tile_tokens[t]) // 16
                    g_t = gp.tile([128, gmax, dh], F16, tag="gath")
                    ix = ixp.tile([128, cols_max], I16, tag="idx")
                    nc.sync.dma_start(ix[:, :cols_t], idx_d[:, col0:col0 + cols_t])

                    # gathers (per chunk, split at GCAP tokens)
                    grp = 0
                    col = 0
                    base_grp = []  # first group index of each chunk's base run
                    ovf_grps = []  # (group, w_i) for overflow windows
                    w_tmp = w_i0
                    for cc in range(NCHUNK):
                        seg = int(p.seg[t, cc])
                        for off in range(0, seg, GCAP):
                            sub = min(GCAP, seg - off)
                            nc.gpsimd.dma_gather(
                                g_t[:, grp + off // WIN:
                                    grp + (off + sub) // WIN, :],
                                table[cc * p.CHUNK:(cc + 1) * p.CHUNK, :],
                                ix[:, col + off // 16:col + (off + sub) // 16],
                                sub, sub, dh,
                                queue_num=qctr[0] % NQ,
                                single_packet=False,
                            )
                            qctr[0] += 1
                        base_grp.append(grp)
                        nbase = (p.TILE_D * SLOTS) // WIN
                        for j in range(int(p.nw[t, cc])):
                            ovf_grps.append((grp + nbase + j, w_tmp))
                            w_tmp += 1
                        grp += seg // WIN
                        col += seg // 16
                    w_i0 = w_tmp
                    col0 += cols_t

                    if layer == 1:
                        # --- node-major accumulation ---
                        # overflow first: feature-major side accumulator
                        accf = None
                        if ovf_grps:
                            accf = psF.tile([128, p.TILE_D], F32, space="PSUM")
                            for si, (g, w_i) in enumerate(ovf_grps):
                                oh = one_hot(w_i)
                                nc.tensor.matmul(accf[:], g_t[:, g, :], oh[:],
                                                 start=(si == 0),
                                                 stop=(si == len(ovf_grps) - 1))
                            accf_sb = midp.tile([128, p.TILE_D], F32, tag="af")
                            nc.scalar.copy(accf_sb[:], accf[:])

                        # node-major base + transpose-accumulated overflow
                        acc = psB.tile([128, 4, dh], F32, space="PSUM")
                        for b in range(4):
                            nmm = 4 * NCHUNK + (1 if ovf_grps else 0)
                            mi = 0
                            for j in range(4):
                                for cc in range(NCHUNK):
                                    nc.tensor.matmul(
                                        acc[:, b, :],
                                        onesb4_sb[:, j, :],
                                        g_t[:, base_grp[cc] + 4 * b + j, :],
                                        start=(mi == 0), stop=(mi == nmm - 1))
                                    mi += 1
                            if ovf_grps:
                                nc.tensor.matmul(
                                    acc[:, b, :],
                                    accf_sb[:, b * 128:(b + 1) * 128],
                                    ident_sb[:],
                                    is_transpose=True,
                                    start=False, stop=True)

                        # G = relu((Z1*norm + b1)) * norm   (node-major)
                        th = midp.tile([128, 4, dh], F32, tag="h")
                        th2 = midp.tile([128, 4, dh], F32, tag="h2")
                        gsb = ytp.tile([128, 4, dh], F16)
                        for b in range(4):
                            nbw = min(128, nd - b * 128)
                            if nbw <= 0:
                                break
                            nck = normc_sb[:, t * 4 + b:t * 4 + b + 1]
                            nc.vector.tensor_scalar(
                                out=th[:, b, :], in0=acc[:, b, :],
                                scalar1=nck, scalar2=None,
                                op0=mybir.AluOpType.mult)
                            nc.vector.tensor_tensor(
                                out=th2[:, b, :], in0=th[:, b, :],
                                in1=b1bc_sb[:], op=mybir.AluOpType.add)
                            nc.scalar.activation(
                                gsb[:, b, :], th2[:, b, :],
                                mybir.ActivationFunctionType.Relu,
                                scale=nck)
                            nc.sync.dma_start(
                                g_loc[d0 + b * 128:d0 + b * 128 + nbw, :],
                                gsb[:nbw, b, :])
                    else:
                        # --- feature-major accumulation (as v2) ---
                        nb = nbp.tile([128, p.TILE_D], F32)
                        nc.sync.dma_start(nb[:], normb_d[:, d0:d0 + p.TILE_D])
                        acc = psB.tile([128, p.TILE_D], F32, space="PSUM")
                        spec = []
                        for cc in range(NCHUNK):
                            nbase = (p.TILE_D * SLOTS) // WIN
                            for w in range(nbase):
                                spec.append((base_grp[cc] + w, "base", w))
                        for g, w_i in ovf_grps:
                            spec.append((g, "ovf", w_i))
                        for si, (g, kind, info) in enumerate(spec):
                            start = si == 0
                            stop = si == len(spec) - 1
                            if kind == "base":
                                nc.tensor.matmul(
                                    acc[:, info * WPD:(info + 1) * WPD],
                                    g_t[:, g, :], onesb_sb[:],
                                    start=start, stop=stop)
                            else:
                                oh = one_hot(info)
                                nc.tensor.matmul(acc[:], g_t[:, g, :], oh[:],
                                                 start=start, stop=stop)

                        a2 = midp.tile([128, p.TILE_D], F16, tag="a2")
                        nc.scalar.copy(a2[:], acc[:])
                        ps2 = psC.tile([128, p.TILE_D], F32, space="PSUM")
                        nc.tensor.matmul(ps2[:], w2_sb[:], a2[:],
                                         start=True, stop=True)
                        h = midp.tile([128, p.TILE_D], F32, tag="h")
                        nc.vector.tensor_tensor(out=h[:], in0=ps2[:], in1=nb[:],
                                                op=mybir.AluOpType.mult)
                        h2 = midp.tile([128, p.TILE_D], F16, tag="hr")
                        nc.scalar.activation(h2[:], h[:],
                                             mybir.ActivationFunctionType.Relu,
                                             bias=b2_sb[:, 0:1])
                        ps3 = psD.tile([dout, p.TILE_D], F32, space="PSUM")
                        nc.tensor.matmul(ps3[:], w3t_sb[:], h2[:],
                                         start=True, stop=True)
                        ot = midp.tile([dout, p.TILE_D], F32, tag="ot")
                        nc.vector.tensor_scalar(
                            out=ot[:], in0=ps3[:], scalar1=b3_sb[:, 0:1],
                            scalar2=None, op0=mybir.AluOpType.add)
                        nc.sync.dma_start(out_d[:, d0:d0 + p.TILE_D], ot[:])

            agg_layer(t_y, layer=1)
            nc.sync.dma_start(g_loc[p.NLOC:p.CONTRIB, :], zeros_sb[:p.ZPAD, :])
            nc.gpsimd.collective_compute(
                "AllGather", mybir.AluOpType.bypass,
                ins=[g_loc.ap().opt()], outs=[t_g.ap().opt()],
                replica_groups=[list(range(NC_CORES))],
            )
            agg_layer(t_g, layer=2)

    nc.compile()
    return nc


def make_in_maps(plan: Plan, features, w1, b1, w2, b2, w3, b3):
    p = plan
    iota, onesb, onesb4, ident = p.consts()
    feats = np.asarray(features, np.float32)
    xs = feats * p.norm[:, None]  # fold the pre-matmul norm scale into X
    b1 = np.asarray(b1, np.float32)
    in_maps = []
    for c in range(NC_CORES):
        in_maps.append(dict(
            featsT=np.ascontiguousarray(
                xs[c * p.NLOC:(c + 1) * p.NLOC].T).astype(np.float16),
            w1=np.ascontiguousarray(w1).astype(np.float16),
            w2=np.ascontiguousarray(w2).astype(np.float16),
            w3t=np.ascontiguousarray(np.asarray(w3).T).astype(np.float16),
            b1bc=np.broadcast_to(b1[None, :], (128, b1.shape[0])).copy(),
            b2=np.asarray(b2, np.float32).reshape(-1, 1),
            b3=np.asarray(b3, np.float32).reshape(-1, 1),
            normb=p.normb[c],
            normc=p.normc[c],
            idx=p.idx[c],
            dstloc=p.dstloc[c],
            negd=-p.dstloc[c],
            iota=iota,
            onesb=onesb,
            onesb4=onesb4,
            ident=ident,
        ))
    return in_maps


def assemble_output(plan: Plan, results, dout):
    p = plan
    h = np.empty((p.N, dout), np.float32)
    for c in range(NC_CORES):
        h[c * p.NLOC:(c + 1) * p.NLOC] = results[c]["outT"][:, :p.NLOC].T
    return h


def run_graphconv(n_nodes, e_subgraph, features, w1, b1, w2, b2, w3, b3,
                  tile_d=512, mode="hw", trace=False):
    plan = Plan(n_nodes, e_subgraph, tile_d=tile_d)
    nc = build_nc(plan, features.shape[1], w1.shape[1], w3.shape[0])
    in_maps = make_in_maps(plan, features, w1, b1, w2, b2, w3, b3)
    if mode == "sim":
        from concourse import bass_interp
        sim = bass_interp.MultiCoreSim(nc, num_cores=NC_CORES)
        for c in range(NC_CORES):
            for k, v in in_maps[c].items():
                sim.cores[c].tensor(k)[:] = v
        sim.simulate(check_with_hw=False)
        results = [{"outT": sim.cores[c].mem_tensor("outT")}
                   for c in range(NC_CORES)]
        res = None
    else:
        res = bass_utils.run_bass_kernel_spmd(
            nc, in_maps, list(range(NC_CORES)), trace=trace)
        results = res.results
    h = assemble_output(plan, results, w3.shape[0])
    return h, res


def kernel(n_subgraph, e_subgraph, to_fetch, features, w1, b1, w2, b2, w3, b3):
    h, _ = run_graphconv(
        n_subgraph.shape[0], e_subgraph, features, w1, b1, w2, b2, w3, b3)
    return (h, h)
